# revision 7
# baseline (speedup 1.0000x reference)
"""Trainium2 Bass kernel for the DeepFermi deconvolution GD problem.

Reference computation (see problem statement): 10 fixed-step GD iterations on
a per-pixel objective

    F(eta) = ||ctc_dc - conv(aif_os, fermi_ir(eta))[::8]/8||^2 / C_dc
             + softplus(lambda) * ||(eta - eta_nn)||^2_Cnn + ||relu(-eta)||^2

The time-axis convolution with the (input-derived, iteration-independent) AIF
is a fixed 64x512 matrix M2; its transpose-products give all needed
reductions:

    s1    = sigmoid(k*(t0 - tsh))                 per pixel, [512]
    q     = M2 @ s1;   ctc_est = A*q
    r2    = (2/C_dc) * (A*q - ctc_dc)             [64]
    gA    = sum_j r2*q
    sd    = s1*(1-s1)
    U     = sum_j r2*(M2 @ sd);  V = sum_j r2*(M2V @ sd)   (M2V = M2*tsh)
    gk    = A*(t0*U - V);  gt0 = A*k*U

All pixels are independent; H(=128 rows) is sharded over the 8 cores, 16 rows
(2048 pixels) per core.  On-chip layout: pixels on partitions (one SBUF tile =
128 pixels x 512 time), sigmoid via one ScalarE activation with per-partition
scale/bias, PE transposes to feed the fixed-matrix matmuls, fused DVE
multiply-reduce ops for the dot products.
"""

import numpy as np

OSAMP = 8
MAX_ITER = 10
NEG_SHIFT = 2 * OSAMP
OTP = 5
C_SHARP = 500.0
LR = 0.1
T = 64
TOS = OSAMP * T  # 512
H = 128
W = 128
N_CORES = 8
ROWS_PER_CORE = H // N_CORES  # 16
TILES = ROWS_PER_CORE  # one 128-pixel tile per local H row
P = 128  # partitions


# ---------------------------------------------------------------------------
# host-side math (iteration independent; exact mirror of the reference's
# jax.image.resize 'linear' semantics)
# ---------------------------------------------------------------------------

def _resize_mat(in_size, out_size):
    """Column-stochastic linear-interp matrix [in, out] matching
    jax.image.resize(method='linear') for upsampling (antialias inactive)."""
    scale = out_size / in_size
    sample_f = (np.arange(out_size) + 0.5) / scale - 0.5
    x = np.abs(sample_f[None, :] - np.arange(in_size)[:, None])
    w = np.maximum(0.0, 1.0 - x)
    tot = w.sum(0, keepdims=True)
    w = np.where(np.abs(tot) > 1e-4, w / tot, 0.0)
    return w  # float64


def _sigmoid(x):
    return 1.0 / (1.0 + np.exp(-x))


def _preprocess(ctc, aif, time, eta_nn, lambda_reg):
    f64 = np.float64
    R = _resize_mat(T, TOS)
    aif0 = (aif.astype(f64) - aif.astype(f64)[..., :OTP].mean(-1, keepdims=True))
    ctc0 = (ctc.astype(f64) - ctc.astype(f64)[..., :OTP].mean(-1, keepdims=True))
    aif_os = (aif0 @ R)[0, 0, 0]                    # [512]
    t_os = time.astype(f64) @ R                     # [512]
    ctc_dc = (ctc0 @ R[:, ::OSAMP])[0]              # [H,W,64]
    C_dc = float((ctc_dc.astype(np.float32) ** 2).sum(dtype=np.float64))
    tsh = t_os - t_os[NEG_SHIFT]
    # fp32-faithful sharp step (saturates exactly like the fp32 reference)
    s2 = _sigmoid((C_SHARP * tsh).astype(np.float32).astype(f64))
    idx = NEG_SHIFT + 8 * np.arange(T)[:, None] - np.arange(TOS)[None, :]
    valid = (idx >= 0) & (idx <= TOS - 1)
    M = np.where(valid, aif_os[np.clip(idx, 0, TOS - 1)], 0.0) / OSAMP  # [64,512]
    M2 = M * s2[None, :]
    M2V = M2 * tsh[None, :]
    C_nn = (eta_nn.astype(f64) ** 2).sum(axis=(0, 2, 3))  # [3]
    sp_lam = np.logaddexp(0.0, float(lambda_reg.reshape(-1)[0]))
    creg = 2.0 * sp_lam / C_nn                      # [3]
    return M2, M2V, tsh, ctc_dc, C_dc, creg


# ---------------------------------------------------------------------------
# bass module (input-value independent; all data arrives via DRAM tensors)
# ---------------------------------------------------------------------------

_NC_CACHE = {}


def _build_nc():
    if "nc" in _NC_CACHE:
        return _NC_CACHE["nc"]

    import concourse.mybir as mybir
    import concourse.tile as tile
    from concourse import bacc

    dt = mybir.dt.float32
    Alu = mybir.AluOpType
    Act = mybir.ActivationFunctionType

    nc = bacc.Bacc("TRN2", target_bir_lowering=False, debug=False)

    # shared constants (identical on every core)
    d_tsh = nc.declare_dram_parameter("tsh_b", [P, TOS], dt, isOutput=False)
    d_ident = nc.declare_dram_parameter("ident", [P, P], dt, isOutput=False)
    d_m2t = nc.declare_dram_parameter("m2t", [P, 4 * T], dt, isOutput=False)
    d_muv = nc.declare_dram_parameter("muv", [P, 4 * 2 * T], dt, isOutput=False)
    # per-core data
    d_nctc = nc.declare_dram_parameter("negctc2", [P, TILES * T], dt, isOutput=False)
    d_eta0 = nc.declare_dram_parameter("eta0", [P, 3 * TILES], dt, isOutput=False)
    d_cpa = nc.declare_dram_parameter("cpa", [P, TILES], dt, isOutput=False)
    d_cpl = nc.declare_dram_parameter("cpl", [P, 2 * TILES], dt, isOutput=False)
    d_consts = nc.declare_dram_parameter("consts", [P, TILES + 3], dt, isOutput=False)
    d_out = nc.declare_dram_parameter("out", [P, 3 * TILES], dt, isOutput=True)

    with tile.TileContext(nc) as tc:
        with (
            tc.tile_pool(name="const", bufs=1) as cpool,
            tc.tile_pool(name="state", bufs=2) as spool,
            tc.tile_pool(name="work", bufs=3) as wpool,
            tc.tile_pool(name="tpose", bufs=2) as tpool,
            tc.tile_pool(name="small", bufs=2) as mpool,
            tc.tile_pool(name="ps_t", bufs=2, space="PSUM") as ps_t,
            tc.tile_pool(name="ps_q", bufs=3, space="PSUM") as ps_q,
        ):
            # ---- load constants ----
            tsh_b = cpool.tile([P, TOS], dt, tag="tsh")
            nc.gpsimd.dma_start(tsh_b[:], d_tsh[:])
            ident = cpool.tile([P, P], dt, tag="ident")
            nc.gpsimd.dma_start(ident[:], d_ident[:])
            m2t = cpool.tile([P, 4 * T], dt, tag="m2t")
            nc.gpsimd.dma_start(m2t[:], d_m2t[:])
            muv = cpool.tile([P, 8 * T], dt, tag="muv")
            nc.gpsimd.dma_start(muv[:], d_muv[:])
            nctc = cpool.tile([P, TILES * T], dt, tag="nctc")
            nc.gpsimd.dma_start(nctc[:], d_nctc[:])
            cpa = cpool.tile([P, TILES], dt, tag="cpa")
            nc.gpsimd.dma_start(cpa[:], d_cpa[:])
            cpl = cpool.tile([P, 2 * TILES], dt, tag="cpl")
            nc.gpsimd.dma_start(cpl[:], d_cpl[:])
            consts = cpool.tile([P, TILES + 3], dt, tag="consts")
            nc.gpsimd.dma_start(consts[:], d_consts[:])
            eta_in = cpool.tile([P, 3 * TILES], dt, tag="eta_in")
            nc.gpsimd.dma_start(eta_in[:], d_eta0[:])

            toc16 = consts[:, 0:TILES]
            sA_ap = consts[:, TILES:TILES + 1]
            sK_ap = consts[:, TILES + 1:TILES + 2]
            sT_ap = consts[:, TILES + 2:TILES + 3]
            cpKL = cpl[:, 0:TILES]
            cpT0L = cpl[:, TILES: 2 * TILES]

            # ---- initial eta state + derived tiles ----
            etaA = spool.tile([P, TILES], dt, tag="etaA")
            etaK = spool.tile([P, TILES], dt, tag="etaK")
            etaT0 = spool.tile([P, TILES], dt, tag="etaT0")
            nc.vector.tensor_copy(etaA[:], eta_in[:, 0:TILES])
            nc.vector.tensor_copy(etaK[:], eta_in[:, TILES: 2 * TILES])
            nc.vector.tensor_copy(etaT0[:], eta_in[:, 2 * TILES: 3 * TILES])

            def make_derived(eA, eK, eT):
                negk = spool.tile([P, TILES], dt, tag="negk")
                nc.vector.tensor_scalar_mul(negk[:], eK[:], -1.0)
                kt0 = spool.tile([P, TILES], dt, tag="kt0")
                nc.vector.tensor_tensor(kt0[:], eK[:], eT[:], Alu.mult)
                a2c = spool.tile([P, TILES], dt, tag="a2c")
                nc.vector.tensor_tensor(a2c[:], eA[:], toc16, Alu.mult)
                return negk, kt0, a2c

            negk, kt0, a2c = make_derived(etaA, etaK, etaT0)

            for it in range(MAX_ITER):
                accGA = mpool.tile([P, TILES], dt, tag="accGA")
                accU = mpool.tile([P, TILES], dt, tag="accU")
                accV = mpool.tile([P, TILES], dt, tag="accV")

                for t in range(TILES):
                    # s1 = sigmoid(-k*tsh + k*t0)   [128 pixels, 512]
                    s1 = wpool.tile([P, TOS], dt, tag="s1")
                    nc.scalar.activation(
                        s1[:], tsh_b[:], Act.Sigmoid,
                        bias=kt0[:, t:t + 1], scale=negk[:, t:t + 1],
                    )
                    # sd = s1*(1-s1)
                    sd = wpool.tile([P, TOS], dt, tag="sd")
                    sdacc = wpool.tile([P, 1], dt, tag="sdacc")
                    nc.vector.affine_mul_reduce(
                        sd[:], sdacc[:], s1[:], s1[:], -1.0, 1.0,
                    )
                    # transpose both into one PSUM tile [128, 1024]
                    trp = ps_t.tile([P, 2 * TOS], dt, tag="trp")
                    for c in range(4):
                        nc.tensor.transpose(
                            trp[:, c * P:(c + 1) * P], s1[:, c * P:(c + 1) * P],
                            ident[:],
                        )
                    for c in range(4):
                        nc.tensor.transpose(
                            trp[:, TOS + c * P: TOS + (c + 1) * P],
                            sd[:, c * P:(c + 1) * P], ident[:],
                        )
                    tr = tpool.tile([P, 2 * TOS], dt, tag="tr")
                    nc.scalar.copy(tr[:], trp[:])
                    s1T = tr[:, 0:TOS]
                    sdT = tr[:, TOS: 2 * TOS]

                    # q = M2 @ s1 -> [128p, 64]; qd|qdv = (M2|M2V) @ sd -> [128p, 128]
                    qq = ps_q.tile([P, 3 * T], dt, tag="qq")
                    for c in range(4):
                        nc.tensor.matmul(
                            qq[:, 0:T], s1T[:, c * P:(c + 1) * P],
                            m2t[:, c * T:(c + 1) * T],
                            start=(c == 0), stop=(c == 3),
                        )
                    for c in range(4):
                        nc.tensor.matmul(
                            qq[:, T: 3 * T], sdT[:, c * P:(c + 1) * P],
                            muv[:, c * 2 * T:(c + 1) * 2 * T],
                            start=(c == 0), stop=(c == 3),
                        )
                    q_ap = qq[:, 0:T]
                    qd_ap = qq[:, T: 2 * T]
                    qdv_ap = qq[:, 2 * T: 3 * T]

                    # r2 = (2A/C_dc)*q - (2/C_dc)*ctc_dc
                    r2 = wpool.tile([P, T], dt, tag="r2")
                    nc.vector.affine_then_add(
                        r2[:], q_ap, nctc[:, t * T:(t + 1) * T],
                        a2c[:, t:t + 1], 0.0,
                    )
                    # dots: accGA[:,t] = sum r2*q ; accU ; accV   (seed 0;
                    # the cpa prior-term is added during the combine phase)
                    dsc = wpool.tile([P, 3 * T], dt, tag="dsc")
                    nc.vector.affine_mul_reduce(
                        dsc[:, 0:T], accGA[:, t:t + 1], q_ap, r2[:], 1.0, 0.0)
                    nc.vector.affine_mul_reduce(
                        dsc[:, T: 2 * T], accU[:, t:t + 1], qd_ap, r2[:], 1.0, 0.0)
                    nc.vector.affine_mul_reduce(
                        dsc[:, 2 * T: 3 * T], accV[:, t:t + 1], qdv_ap, r2[:], 1.0, 0.0)

                # ---- combine: eta <- eta - LR*grad (FD=16 ops) ----
                mA = mpool.tile([P, TILES], dt, tag="mA")
                nc.vector.tensor_scalar(mA[:], etaA[:], 0.0, -2.0 * LR,
                                        Alu.min, Alu.mult)
                ga2 = mpool.tile([P, TILES], dt, tag="ga2")
                nc.vector.tensor_tensor(ga2[:], accGA[:], cpa[:], Alu.add)
                tA = mpool.tile([P, TILES], dt, tag="tA")
                nc.vector.affine_then_add(tA[:], ga2[:], mA[:], -LR, 0.0)
                etaA2 = spool.tile([P, TILES], dt, tag="etaA")
                nc.vector.affine_then_add(etaA2[:], etaA[:], tA[:], sA_ap, 0.0)

                p1 = mpool.tile([P, TILES], dt, tag="p1")
                nc.vector.tensor_tensor(p1[:], etaA[:], accU[:], Alu.mult)
                p2 = mpool.tile([P, TILES], dt, tag="p2")
                nc.vector.tensor_tensor(p2[:], etaA[:], accV[:], Alu.mult)
                wk = mpool.tile([P, TILES], dt, tag="wk")
                nc.vector.tensor_tensor(wk[:], etaT0[:], p1[:], Alu.mult)
                zk = mpool.tile([P, TILES], dt, tag="zk")
                nc.vector.tensor_tensor(zk[:], wk[:], p2[:], Alu.subtract)
                mK = mpool.tile([P, TILES], dt, tag="mK")
                nc.vector.tensor_scalar(mK[:], etaK[:], 0.0, -2.0 * LR,
                                        Alu.min, Alu.mult)
                tK = mpool.tile([P, TILES], dt, tag="tK")
                nc.vector.affine_then_add(tK[:], zk[:], mK[:], -LR, 0.0)
                tK2 = mpool.tile([P, TILES], dt, tag="tK2")
                nc.vector.tensor_tensor(tK2[:], tK[:], cpKL, Alu.add)
                etaK2 = spool.tile([P, TILES], dt, tag="etaK")
                nc.vector.affine_then_add(etaK2[:], etaK[:], tK2[:], sK_ap, 0.0)

                w2 = mpool.tile([P, TILES], dt, tag="w2")
                nc.vector.tensor_tensor(w2[:], p1[:], etaK[:], Alu.mult)
                mT = mpool.tile([P, TILES], dt, tag="mT")
                nc.vector.tensor_scalar(mT[:], etaT0[:], 0.0, -2.0 * LR,
                                        Alu.min, Alu.mult)
                tT = mpool.tile([P, TILES], dt, tag="tT")
                nc.vector.affine_then_add(tT[:], w2[:], mT[:], -LR, 0.0)
                tT2 = mpool.tile([P, TILES], dt, tag="tT2")
                nc.vector.tensor_tensor(tT2[:], tT[:], cpT0L, Alu.add)
                etaT02 = spool.tile([P, TILES], dt, tag="etaT0")
                nc.vector.affine_then_add(etaT02[:], etaT0[:], tT2[:], sT_ap, 0.0)

                etaA, etaK, etaT0 = etaA2, etaK2, etaT02
                if it < MAX_ITER - 1:
                    negk, kt0, a2c = make_derived(etaA, etaK, etaT0)

            nc.gpsimd.dma_start(d_out[:, 0:TILES], etaA[:])
            nc.gpsimd.dma_start(d_out[:, TILES: 2 * TILES], etaK[:])
            nc.gpsimd.dma_start(d_out[:, 2 * TILES: 3 * TILES], etaT0[:])

    nc.finalize()
    _NC_CACHE["nc"] = nc
    return nc


# ---------------------------------------------------------------------------
# public entry point
# ---------------------------------------------------------------------------

def _make_in_maps(ctc, aif, time, eta_nn, lambda_reg):
    f32 = np.float32
    M2, M2V, tsh, ctc_dc, C_dc, creg = _preprocess(ctc, aif, time, eta_nn, lambda_reg)

    toc = 2.0 / C_dc
    sA, sK, sT0 = (1.0 - LR * creg).astype(np.float64)

    tsh_b = np.broadcast_to(tsh.astype(f32), (P, TOS)).copy()
    ident = np.eye(P, dtype=f32)
    # m2t[vv, 64c+j] = M2[j, 128c+vv];  muv[vv, 128c+j'] = (M2|M2V)[j', 128c+vv]
    m2t = np.zeros((P, 4 * T), f32)
    muv = np.zeros((P, 8 * T), f32)
    for c in range(4):
        blk = M2[:, c * P:(c + 1) * P]       # [64,128]
        blkv = M2V[:, c * P:(c + 1) * P]
        m2t[:, c * T:(c + 1) * T] = blk.T
        muv[:, c * 2 * T: c * 2 * T + T] = blk.T
        muv[:, c * 2 * T + T: (c + 1) * 2 * T] = blkv.T

    consts = np.zeros((P, TILES + 3), f32)
    consts[:, 0:TILES] = toc
    consts[:, TILES] = sA
    consts[:, TILES + 1] = sK
    consts[:, TILES + 2] = sT0

    in_maps = []
    for m in range(N_CORES):
        rows = slice(m * ROWS_PER_CORE, (m + 1) * ROWS_PER_CORE)
        # ctc_dc[h, w, j]: tile t = local row, partition p = w
        cd = ctc_dc[rows]                     # [16, 128, 64]
        negctc2 = np.ascontiguousarray(
            (-toc * cd).transpose(1, 0, 2).reshape(P, TILES * T)).astype(f32)
        pr = eta_nn[0, :, rows, :].astype(np.float64)   # [3, 16, 128] (c, t, p)
        eta0 = np.ascontiguousarray(
            pr.transpose(2, 0, 1).reshape(P, 3 * TILES)).astype(f32)
        cpa = np.ascontiguousarray((-creg[0] * pr[0]).T).astype(f32)  # [128,16]
        cpl = np.zeros((P, 2 * TILES), f32)
        cpl[:, 0:TILES] = (LR * creg[1] * pr[1]).T
        cpl[:, TILES: 2 * TILES] = (LR * creg[2] * pr[2]).T
        in_maps.append({
            "tsh_b": tsh_b, "ident": ident, "m2t": m2t, "muv": muv,
            "negctc2": negctc2, "eta0": eta0, "cpa": cpa, "cpl": cpl,
            "consts": consts,
        })
    return in_maps


def kernel(ctc, aif, time, seg, eta_nn, lambda_reg):
    from concourse.bass_utils import run_bass_kernel_spmd

    ctc = np.asarray(ctc)
    aif = np.asarray(aif)
    time = np.asarray(time)
    eta_nn = np.asarray(eta_nn)
    lambda_reg = np.asarray(lambda_reg)

    in_maps = _make_in_maps(ctc, aif, time, eta_nn, lambda_reg)
    nc = _build_nc()
    res = run_bass_kernel_spmd(nc, in_maps, list(range(N_CORES)))

    out = np.zeros((1, 3, H, W), np.float32)
    for m in range(N_CORES):
        rows = slice(m * ROWS_PER_CORE, (m + 1) * ROWS_PER_CORE)
        arr = res.results[m]["out"]                  # [128, 48]
        out[0, :, rows, :] = arr.reshape(P, 3, TILES).transpose(1, 2, 0)
    return out


# revision 9
# speedup vs baseline: 1.0040x; 1.0040x over previous
"""Trainium2 Bass kernel for the DeepFermi deconvolution GD problem.

Reference computation (see problem statement): 10 fixed-step GD iterations on
a per-pixel objective

    F(eta) = ||ctc_dc - conv(aif_os, fermi_ir(eta))[::8]/8||^2 / C_dc
             + softplus(lambda) * ||(eta - eta_nn)||^2_Cnn + ||relu(-eta)||^2

The time-axis convolution with the (input-derived, iteration-independent) AIF
is a fixed 64x512 matrix M2; its transpose-products give all needed
reductions:

    s1    = sigmoid(k*(t0 - tsh))                 per pixel, [512]
    q     = M2 @ s1;   ctc_est = A*q
    r2    = (2/C_dc) * (A*q - ctc_dc)             [64]
    gA    = sum_j r2*q
    sd    = s1*(1-s1)
    U     = sum_j r2*(M2 @ sd);  V = sum_j r2*(M2V @ sd)   (M2V = M2*tsh)
    gk    = A*(t0*U - V);  gt0 = A*k*U

All pixels are independent; H(=128 rows) is sharded over the 8 cores, 16 rows
(2048 pixels) per core.  On-chip layout: pixels on partitions (one SBUF tile =
128 pixels x 512 time), sigmoid via one ScalarE activation with per-partition
scale/bias, PE transposes to feed the fixed-matrix matmuls, fused DVE
multiply-reduce ops for the dot products.
"""

import numpy as np

OSAMP = 8
MAX_ITER = 10
NEG_SHIFT = 2 * OSAMP
OTP = 5
C_SHARP = 500.0
LR = 0.1
T = 64
TOS = OSAMP * T  # 512
H = 128
W = 128
N_CORES = 8
ROWS_PER_CORE = H // N_CORES  # 16
TILES = ROWS_PER_CORE  # one 128-pixel tile per local H row
P = 128  # partitions


# ---------------------------------------------------------------------------
# host-side math (iteration independent; exact mirror of the reference's
# jax.image.resize 'linear' semantics)
# ---------------------------------------------------------------------------

def _resize_mat(in_size, out_size):
    """Column-stochastic linear-interp matrix [in, out] matching
    jax.image.resize(method='linear') for upsampling (antialias inactive)."""
    scale = out_size / in_size
    sample_f = (np.arange(out_size) + 0.5) / scale - 0.5
    x = np.abs(sample_f[None, :] - np.arange(in_size)[:, None])
    w = np.maximum(0.0, 1.0 - x)
    tot = w.sum(0, keepdims=True)
    w = np.where(np.abs(tot) > 1e-4, w / tot, 0.0)
    return w  # float64


def _sigmoid(x):
    return 1.0 / (1.0 + np.exp(-x))


def _preprocess(ctc, aif, time, eta_nn, lambda_reg):
    f64 = np.float64
    R = _resize_mat(T, TOS)
    aif0 = (aif.astype(f64) - aif.astype(f64)[..., :OTP].mean(-1, keepdims=True))
    ctc0 = (ctc.astype(f64) - ctc.astype(f64)[..., :OTP].mean(-1, keepdims=True))
    aif_os = (aif0 @ R)[0, 0, 0]                    # [512]
    t_os = time.astype(f64) @ R                     # [512]
    ctc_dc = (ctc0 @ R[:, ::OSAMP])[0]              # [H,W,64]
    C_dc = float((ctc_dc.astype(np.float32) ** 2).sum(dtype=np.float64))
    tsh = t_os - t_os[NEG_SHIFT]
    # fp32-faithful sharp step (saturates exactly like the fp32 reference)
    s2 = _sigmoid((C_SHARP * tsh).astype(np.float32).astype(f64))
    idx = NEG_SHIFT + 8 * np.arange(T)[:, None] - np.arange(TOS)[None, :]
    valid = (idx >= 0) & (idx <= TOS - 1)
    M = np.where(valid, aif_os[np.clip(idx, 0, TOS - 1)], 0.0) / OSAMP  # [64,512]
    M2 = M * s2[None, :]
    M2V = M2 * tsh[None, :]
    C_nn = (eta_nn.astype(f64) ** 2).sum(axis=(0, 2, 3))  # [3]
    sp_lam = np.logaddexp(0.0, float(lambda_reg.reshape(-1)[0]))
    creg = 2.0 * sp_lam / C_nn                      # [3]
    return M2, M2V, tsh, ctc_dc, C_dc, creg


# ---------------------------------------------------------------------------
# bass module (input-value independent; all data arrives via DRAM tensors)
# ---------------------------------------------------------------------------

_NC_CACHE = {}


def _build_nc():
    if "nc" in _NC_CACHE:
        return _NC_CACHE["nc"]

    import concourse.mybir as mybir
    import concourse.tile as tile
    from concourse import bacc

    dt = mybir.dt.float32
    bf = mybir.dt.bfloat16
    Alu = mybir.AluOpType
    Act = mybir.ActivationFunctionType

    nc = bacc.Bacc("TRN2", target_bir_lowering=False, debug=False)

    # shared constants (identical on every core)
    d_otsh = nc.declare_dram_parameter("otsh", [2, TOS], bf, isOutput=False)
    d_ident = nc.declare_dram_parameter("ident", [P, P], bf, isOutput=False)
    d_m2t = nc.declare_dram_parameter("m2t", [P, 4 * T], bf, isOutput=False)
    d_muv = nc.declare_dram_parameter("muv", [P, 4 * 2 * T], bf, isOutput=False)
    # per-core data
    d_nctc = nc.declare_dram_parameter("negctc2", [P, TILES * T], dt, isOutput=False)
    d_eta0 = nc.declare_dram_parameter("eta0", [P, 3 * TILES], dt, isOutput=False)
    d_cpa = nc.declare_dram_parameter("cpa", [P, TILES], dt, isOutput=False)
    d_cpl = nc.declare_dram_parameter("cpl", [P, 2 * TILES], dt, isOutput=False)
    d_consts = nc.declare_dram_parameter("consts", [P, TILES + 3], dt, isOutput=False)
    d_out = nc.declare_dram_parameter("out", [P, 3 * TILES], dt, isOutput=True)

    with tile.TileContext(nc) as tc:
        with (
            tc.tile_pool(name="const", bufs=1) as cpool,
            tc.tile_pool(name="state", bufs=2) as spool,
            tc.tile_pool(name="work", bufs=3) as wpool,
            tc.tile_pool(name="tpose", bufs=2) as tpool,
            tc.tile_pool(name="small", bufs=2) as mpool,
            tc.tile_pool(name="ps_t", bufs=2, space="PSUM") as ps_t,
            tc.tile_pool(name="ps_q", bufs=3, space="PSUM") as ps_q,
            tc.tile_pool(name="ps_k", bufs=2, space="PSUM") as ps_k,
        ):
            # ---- load constants ----
            otsh = cpool.tile([2, TOS], bf, tag="otsh")
            nc.gpsimd.dma_start(otsh[:], d_otsh[:])
            ident = cpool.tile([P, P], bf, tag="ident")
            nc.gpsimd.dma_start(ident[:], d_ident[:])
            m2t = cpool.tile([P, 4 * T], bf, tag="m2t")
            nc.gpsimd.dma_start(m2t[:], d_m2t[:])
            muv = cpool.tile([P, 8 * T], bf, tag="muv")
            nc.gpsimd.dma_start(muv[:], d_muv[:])
            nctc = cpool.tile([P, TILES * T], dt, tag="nctc")
            nc.gpsimd.dma_start(nctc[:], d_nctc[:])
            cpa = cpool.tile([P, TILES], dt, tag="cpa")
            nc.gpsimd.dma_start(cpa[:], d_cpa[:])
            cpl = cpool.tile([P, 2 * TILES], dt, tag="cpl")
            nc.gpsimd.dma_start(cpl[:], d_cpl[:])
            consts = cpool.tile([P, TILES + 3], dt, tag="consts")
            nc.gpsimd.dma_start(consts[:], d_consts[:])
            eta_in = cpool.tile([P, 3 * TILES], dt, tag="eta_in")
            nc.gpsimd.dma_start(eta_in[:], d_eta0[:])

            toc16 = consts[:, 0:TILES]
            sA_ap = consts[:, TILES:TILES + 1]
            sK_ap = consts[:, TILES + 1:TILES + 2]
            sT_ap = consts[:, TILES + 2:TILES + 3]
            cpKL = cpl[:, 0:TILES]
            cpT0L = cpl[:, TILES: 2 * TILES]

            # ---- initial eta state + derived tiles ----
            etaA = spool.tile([P, TILES], dt, tag="etaA")
            etaK = spool.tile([P, TILES], dt, tag="etaK")
            etaT0 = spool.tile([P, TILES], dt, tag="etaT0")
            nc.vector.tensor_copy(etaA[:], eta_in[:, 0:TILES])
            nc.vector.tensor_copy(etaK[:], eta_in[:, TILES: 2 * TILES])
            nc.vector.tensor_copy(etaT0[:], eta_in[:, 2 * TILES: 3 * TILES])

            def make_derived(eA, eK, eT):
                # kn[:, 2t] = (k*t0)_t, kn[:, 2t+1] = (-k)_t  (bf16), then
                # transpose so tile t's arg-matmul rhs is knT[2t:2t+2, :]
                kn = spool.tile([P, 2 * TILES], bf, tag="kn")
                nc.vector.tensor_tensor(kn[:, 0:2 * TILES:2], eK[:], eT[:],
                                        Alu.mult)
                nc.vector.tensor_scalar_mul(kn[:, 1:2 * TILES:2], eK[:], -1.0)
                knt_ps = ps_k.tile([2 * TILES, P], bf, tag="kntp")
                nc.tensor.transpose(knt_ps[:], kn[:], ident[:])
                knT = spool.tile([2 * TILES, P], bf, tag="knT")
                nc.scalar.copy(knT[:], knt_ps[:])
                a2c = spool.tile([P, TILES], dt, tag="a2c")
                nc.vector.tensor_tensor(a2c[:], eA[:], toc16, Alu.mult)
                return knT, a2c

            knT, a2c = make_derived(etaA, etaK, etaT0)

            for it in range(MAX_ITER):
                accGA = mpool.tile([P, TILES], dt, tag="accGA")
                accU = mpool.tile([P, TILES], dt, tag="accU")
                accV = mpool.tile([P, TILES], dt, tag="accV")

                for t in range(TILES):
                    # argT[v,p] = kt0_p - k_p*tsh_v via rank-2 matmul:
                    # lhsT = [ones; tsh] chunk, rhs = knT[2t:2t+2, :]
                    argp = ps_t.tile([P, TOS], dt, tag="argp")
                    for c in range(4):
                        nc.tensor.matmul(
                            argp[:, c * P:(c + 1) * P],
                            otsh[:, c * P:(c + 1) * P],
                            knT[2 * t:2 * t + 2, :],
                            start=True, stop=True,
                        )
                    # s1T = sigmoid(argT)  (PSUM -> SBUF, bf16 out)
                    s1T = wpool.tile([P, TOS], bf, tag="s1T")
                    nc.scalar.activation(s1T[:], argp[:], Act.Sigmoid)
                    # sdT = s1T*(1-s1T)
                    sdT = wpool.tile([P, TOS], bf, tag="sdT")
                    sdacc = wpool.tile([P, 1], dt, tag="sdacc")
                    nc.vector.affine_mul_reduce(
                        sdT[:], sdacc[:], s1T[:], s1T[:], -1.0, 1.0,
                    )

                    # q = M2 @ s1 -> [128p, 64]; qd|qdv = (M2|M2V) @ sd -> [128p, 128]
                    qq = ps_q.tile([P, 3 * T], dt, tag="qq")
                    for c in range(4):
                        nc.tensor.matmul(
                            qq[:, 0:T], s1T[:, c * P:(c + 1) * P],
                            m2t[:, c * T:(c + 1) * T],
                            start=(c == 0), stop=(c == 3),
                        )
                    for c in range(4):
                        nc.tensor.matmul(
                            qq[:, T: 3 * T], sdT[:, c * P:(c + 1) * P],
                            muv[:, c * 2 * T:(c + 1) * 2 * T],
                            start=(c == 0), stop=(c == 3),
                        )
                    # single PSUM->SBUF copy (bf16) for all of q|qd|qdv
                    qqs = wpool.tile([P, 3 * T], bf, tag="qqs")
                    nc.scalar.copy(qqs[:], qq[:])
                    q_ap = qqs[:, 0:T]
                    qd_ap = qqs[:, T: 2 * T]
                    qdv_ap = qqs[:, 2 * T: 3 * T]

                    # r2 = (2A/C_dc)*q - (2/C_dc)*ctc_dc
                    r2 = wpool.tile([P, T], dt, tag="r2")
                    nc.vector.affine_then_add(
                        r2[:], q_ap, nctc[:, t * T:(t + 1) * T],
                        a2c[:, t:t + 1], 0.0,
                    )
                    # dots: accGA[:,t] = sum r2*q ; accU ; accV   (seed 0;
                    # the cpa prior-term is added during the combine phase)
                    dsc = wpool.tile([P, 3 * T], dt, tag="dsc")
                    nc.vector.affine_mul_reduce(
                        dsc[:, 0:T], accGA[:, t:t + 1], q_ap, r2[:], 1.0, 0.0)
                    nc.vector.affine_mul_reduce(
                        dsc[:, T: 2 * T], accU[:, t:t + 1], qd_ap, r2[:], 1.0, 0.0)
                    nc.vector.affine_mul_reduce(
                        dsc[:, 2 * T: 3 * T], accV[:, t:t + 1], qdv_ap, r2[:], 1.0, 0.0)

                # ---- combine: eta <- eta - LR*grad (FD=16 ops) ----
                mA = mpool.tile([P, TILES], dt, tag="mA")
                nc.vector.tensor_scalar(mA[:], etaA[:], 0.0, -2.0 * LR,
                                        Alu.min, Alu.mult)
                ga2 = mpool.tile([P, TILES], dt, tag="ga2")
                nc.vector.tensor_tensor(ga2[:], accGA[:], cpa[:], Alu.add)
                tA = mpool.tile([P, TILES], dt, tag="tA")
                nc.vector.affine_then_add(tA[:], ga2[:], mA[:], -LR, 0.0)
                etaA2 = spool.tile([P, TILES], dt, tag="etaA")
                nc.vector.affine_then_add(etaA2[:], etaA[:], tA[:], sA_ap, 0.0)

                p1 = mpool.tile([P, TILES], dt, tag="p1")
                nc.vector.tensor_tensor(p1[:], etaA[:], accU[:], Alu.mult)
                p2 = mpool.tile([P, TILES], dt, tag="p2")
                nc.vector.tensor_tensor(p2[:], etaA[:], accV[:], Alu.mult)
                wk = mpool.tile([P, TILES], dt, tag="wk")
                nc.vector.tensor_tensor(wk[:], etaT0[:], p1[:], Alu.mult)
                zk = mpool.tile([P, TILES], dt, tag="zk")
                nc.vector.tensor_tensor(zk[:], wk[:], p2[:], Alu.subtract)
                mK = mpool.tile([P, TILES], dt, tag="mK")
                nc.vector.tensor_scalar(mK[:], etaK[:], 0.0, -2.0 * LR,
                                        Alu.min, Alu.mult)
                tK = mpool.tile([P, TILES], dt, tag="tK")
                nc.vector.affine_then_add(tK[:], zk[:], mK[:], -LR, 0.0)
                tK2 = mpool.tile([P, TILES], dt, tag="tK2")
                nc.vector.tensor_tensor(tK2[:], tK[:], cpKL, Alu.add)
                etaK2 = spool.tile([P, TILES], dt, tag="etaK")
                nc.vector.affine_then_add(etaK2[:], etaK[:], tK2[:], sK_ap, 0.0)

                w2 = mpool.tile([P, TILES], dt, tag="w2")
                nc.vector.tensor_tensor(w2[:], p1[:], etaK[:], Alu.mult)
                mT = mpool.tile([P, TILES], dt, tag="mT")
                nc.vector.tensor_scalar(mT[:], etaT0[:], 0.0, -2.0 * LR,
                                        Alu.min, Alu.mult)
                tT = mpool.tile([P, TILES], dt, tag="tT")
                nc.vector.affine_then_add(tT[:], w2[:], mT[:], -LR, 0.0)
                tT2 = mpool.tile([P, TILES], dt, tag="tT2")
                nc.vector.tensor_tensor(tT2[:], tT[:], cpT0L, Alu.add)
                etaT02 = spool.tile([P, TILES], dt, tag="etaT0")
                nc.vector.affine_then_add(etaT02[:], etaT0[:], tT2[:], sT_ap, 0.0)

                etaA, etaK, etaT0 = etaA2, etaK2, etaT02
                if it < MAX_ITER - 1:
                    knT, a2c = make_derived(etaA, etaK, etaT0)

            nc.gpsimd.dma_start(d_out[:, 0:TILES], etaA[:])
            nc.gpsimd.dma_start(d_out[:, TILES: 2 * TILES], etaK[:])
            nc.gpsimd.dma_start(d_out[:, 2 * TILES: 3 * TILES], etaT0[:])

    nc.finalize()
    _NC_CACHE["nc"] = nc
    return nc


# ---------------------------------------------------------------------------
# public entry point
# ---------------------------------------------------------------------------

def _make_in_maps(ctc, aif, time, eta_nn, lambda_reg):
    f32 = np.float32
    M2, M2V, tsh, ctc_dc, C_dc, creg = _preprocess(ctc, aif, time, eta_nn, lambda_reg)

    toc = 2.0 / C_dc
    sA, sK, sT0 = (1.0 - LR * creg).astype(np.float64)

    import ml_dtypes
    bf16 = ml_dtypes.bfloat16
    otsh = np.zeros((2, TOS), bf16)
    otsh[0, :] = 1.0
    otsh[1, :] = tsh.astype(np.float32)
    ident = np.eye(P, dtype=bf16)
    # m2t[vv, 64c+j] = M2[j, 128c+vv];  muv[vv, 128c+j'] = (M2|M2V)[j', 128c+vv]
    m2t = np.zeros((P, 4 * T), bf16)
    muv = np.zeros((P, 8 * T), bf16)
    for c in range(4):
        blk = M2[:, c * P:(c + 1) * P]       # [64,128]
        blkv = M2V[:, c * P:(c + 1) * P]
        m2t[:, c * T:(c + 1) * T] = blk.T
        muv[:, c * 2 * T: c * 2 * T + T] = blk.T
        muv[:, c * 2 * T + T: (c + 1) * 2 * T] = blkv.T

    consts = np.zeros((P, TILES + 3), f32)
    consts[:, 0:TILES] = toc
    consts[:, TILES] = sA
    consts[:, TILES + 1] = sK
    consts[:, TILES + 2] = sT0

    in_maps = []
    for m in range(N_CORES):
        rows = slice(m * ROWS_PER_CORE, (m + 1) * ROWS_PER_CORE)
        # ctc_dc[h, w, j]: tile t = local row, partition p = w
        cd = ctc_dc[rows]                     # [16, 128, 64]
        negctc2 = np.ascontiguousarray(
            (-toc * cd).transpose(1, 0, 2).reshape(P, TILES * T)).astype(f32)
        pr = eta_nn[0, :, rows, :].astype(np.float64)   # [3, 16, 128] (c, t, p)
        eta0 = np.ascontiguousarray(
            pr.transpose(2, 0, 1).reshape(P, 3 * TILES)).astype(f32)
        cpa = np.ascontiguousarray((-creg[0] * pr[0]).T).astype(f32)  # [128,16]
        cpl = np.zeros((P, 2 * TILES), f32)
        cpl[:, 0:TILES] = (LR * creg[1] * pr[1]).T
        cpl[:, TILES: 2 * TILES] = (LR * creg[2] * pr[2]).T
        in_maps.append({
            "otsh": otsh, "ident": ident, "m2t": m2t, "muv": muv,
            "negctc2": negctc2, "eta0": eta0, "cpa": cpa, "cpl": cpl,
            "consts": consts,
        })
    return in_maps


def kernel(ctc, aif, time, seg, eta_nn, lambda_reg):
    from concourse.bass_utils import run_bass_kernel_spmd

    ctc = np.asarray(ctc)
    aif = np.asarray(aif)
    time = np.asarray(time)
    eta_nn = np.asarray(eta_nn)
    lambda_reg = np.asarray(lambda_reg)

    in_maps = _make_in_maps(ctc, aif, time, eta_nn, lambda_reg)
    nc = _build_nc()
    res = run_bass_kernel_spmd(nc, in_maps, list(range(N_CORES)))

    out = np.zeros((1, 3, H, W), np.float32)
    for m in range(N_CORES):
        rows = slice(m * ROWS_PER_CORE, (m + 1) * ROWS_PER_CORE)
        arr = res.results[m]["out"]                  # [128, 48]
        out[0, :, rows, :] = arr.reshape(P, 3, TILES).transpose(1, 2, 0)
    return out


# revision 10
# speedup vs baseline: 1.8883x; 1.8806x over previous
"""Trainium2 Bass kernel for the DeepFermi deconvolution GD problem.

Reference computation (see problem statement): 10 fixed-step GD iterations on
a per-pixel objective

    F(eta) = ||ctc_dc - conv(aif_os, fermi_ir(eta))[::8]/8||^2 / C_dc
             + softplus(lambda) * ||(eta - eta_nn)||^2_Cnn + ||relu(-eta)||^2

The time-axis convolution with the (input-derived, iteration-independent) AIF
is a fixed 64x512 matrix M2; its transpose-products give all needed
reductions:

    s1    = sigmoid(k*(t0 - tsh))                 per pixel, [512]
    q     = M2 @ s1;   ctc_est = A*q
    r2    = (2/C_dc) * (A*q - ctc_dc)             [64]
    gA    = sum_j r2*q
    sd    = s1*(1-s1)
    U     = sum_j r2*(M2 @ sd);  V = sum_j r2*(M2V @ sd)   (M2V = M2*tsh)
    gk    = A*(t0*U - V);  gt0 = A*k*U

All pixels are independent; H(=128 rows) is sharded over the 8 cores, 16 rows
(2048 pixels) per core.  On-chip layout: pixels on partitions (one SBUF tile =
128 pixels x 512 time), sigmoid via one ScalarE activation with per-partition
scale/bias, PE transposes to feed the fixed-matrix matmuls, fused DVE
multiply-reduce ops for the dot products.
"""

import numpy as np

OSAMP = 8
MAX_ITER = 10
NEG_SHIFT = 2 * OSAMP
OTP = 5
C_SHARP = 500.0
LR = 0.1
T = 64
TOS = OSAMP * T  # 512
H = 128
W = 128
N_CORES = 8
ROWS_PER_CORE = H // N_CORES  # 16
TILES = ROWS_PER_CORE  # one 128-pixel tile per local H row
P = 128  # partitions


# ---------------------------------------------------------------------------
# host-side math (iteration independent; exact mirror of the reference's
# jax.image.resize 'linear' semantics)
# ---------------------------------------------------------------------------

def _resize_mat(in_size, out_size):
    """Column-stochastic linear-interp matrix [in, out] matching
    jax.image.resize(method='linear') for upsampling (antialias inactive)."""
    scale = out_size / in_size
    sample_f = (np.arange(out_size) + 0.5) / scale - 0.5
    x = np.abs(sample_f[None, :] - np.arange(in_size)[:, None])
    w = np.maximum(0.0, 1.0 - x)
    tot = w.sum(0, keepdims=True)
    w = np.where(np.abs(tot) > 1e-4, w / tot, 0.0)
    return w  # float64


def _sigmoid(x):
    return 1.0 / (1.0 + np.exp(-x))


def _preprocess(ctc, aif, time, eta_nn, lambda_reg):
    f64 = np.float64
    R = _resize_mat(T, TOS)
    aif0 = (aif.astype(f64) - aif.astype(f64)[..., :OTP].mean(-1, keepdims=True))
    ctc0 = (ctc.astype(f64) - ctc.astype(f64)[..., :OTP].mean(-1, keepdims=True))
    aif_os = (aif0 @ R)[0, 0, 0]                    # [512]
    t_os = time.astype(f64) @ R                     # [512]
    ctc_dc = (ctc0 @ R[:, ::OSAMP])[0]              # [H,W,64]
    C_dc = float((ctc_dc.astype(np.float32) ** 2).sum(dtype=np.float64))
    tsh = t_os - t_os[NEG_SHIFT]
    # fp32-faithful sharp step (saturates exactly like the fp32 reference)
    s2 = _sigmoid((C_SHARP * tsh).astype(np.float32).astype(f64))
    idx = NEG_SHIFT + 8 * np.arange(T)[:, None] - np.arange(TOS)[None, :]
    valid = (idx >= 0) & (idx <= TOS - 1)
    M = np.where(valid, aif_os[np.clip(idx, 0, TOS - 1)], 0.0) / OSAMP  # [64,512]
    M2 = M * s2[None, :]
    M2V = M2 * tsh[None, :]
    C_nn = (eta_nn.astype(f64) ** 2).sum(axis=(0, 2, 3))  # [3]
    sp_lam = np.logaddexp(0.0, float(lambda_reg.reshape(-1)[0]))
    creg = 2.0 * sp_lam / C_nn                      # [3]
    return M2, M2V, tsh, ctc_dc, C_dc, creg


# ---------------------------------------------------------------------------
# bass module (input-value independent; all data arrives via DRAM tensors)
# ---------------------------------------------------------------------------

_NC_CACHE = {}


def _build_nc():
    if "nc" in _NC_CACHE:
        return _NC_CACHE["nc"]

    import concourse.mybir as mybir
    import concourse.tile as tile
    from concourse import bacc

    dt = mybir.dt.float32
    bf = mybir.dt.bfloat16
    Alu = mybir.AluOpType
    Act = mybir.ActivationFunctionType

    nc = bacc.Bacc("TRN2", target_bir_lowering=False, debug=False)

    # shared constants (identical on every core)
    d_argw = nc.declare_dram_parameter("argw", [2 * TILES, 4 * TILES * P], bf,
                                       isOutput=False)
    d_ident = nc.declare_dram_parameter("ident", [P, P], bf, isOutput=False)
    d_m2t = nc.declare_dram_parameter("m2t", [P, 4 * T], bf, isOutput=False)
    d_muv = nc.declare_dram_parameter("muv", [P, 4 * 2 * T], bf, isOutput=False)
    # per-core data
    d_nctc = nc.declare_dram_parameter("negctc2", [P, TILES * T], dt, isOutput=False)
    d_eta0 = nc.declare_dram_parameter("eta0", [P, 3 * TILES], dt, isOutput=False)
    d_cpa = nc.declare_dram_parameter("cpa", [P, TILES], dt, isOutput=False)
    d_cpl = nc.declare_dram_parameter("cpl", [P, 2 * TILES], dt, isOutput=False)
    d_consts = nc.declare_dram_parameter("consts", [P, TILES + 3], dt, isOutput=False)
    d_out = nc.declare_dram_parameter("out", [P, 3 * TILES], dt, isOutput=True)

    with tile.TileContext(nc) as tc:
        with (
            tc.tile_pool(name="const", bufs=1) as cpool,
            tc.tile_pool(name="state", bufs=2) as spool,
            tc.tile_pool(name="work", bufs=3) as wpool,
            tc.tile_pool(name="tpose", bufs=2) as tpool,
            tc.tile_pool(name="small", bufs=2) as mpool,
            tc.tile_pool(name="ps_t", bufs=2, space="PSUM") as ps_t,
            tc.tile_pool(name="ps_q", bufs=3, space="PSUM") as ps_q,
            tc.tile_pool(name="ps_k", bufs=2, space="PSUM") as ps_k,
        ):
            # ---- load constants ----
            argw = cpool.tile([2 * TILES, 4 * TILES * P], bf, tag="argw")
            nc.gpsimd.dma_start(argw[:], d_argw[:])
            ident = cpool.tile([P, P], bf, tag="ident")
            nc.gpsimd.dma_start(ident[:], d_ident[:])
            m2t = cpool.tile([P, 4 * T], bf, tag="m2t")
            nc.gpsimd.dma_start(m2t[:], d_m2t[:])
            muv = cpool.tile([P, 8 * T], bf, tag="muv")
            nc.gpsimd.dma_start(muv[:], d_muv[:])
            nctc = cpool.tile([P, TILES * T], dt, tag="nctc")
            nc.gpsimd.dma_start(nctc[:], d_nctc[:])
            cpa = cpool.tile([P, TILES], dt, tag="cpa")
            nc.gpsimd.dma_start(cpa[:], d_cpa[:])
            cpl = cpool.tile([P, 2 * TILES], dt, tag="cpl")
            nc.gpsimd.dma_start(cpl[:], d_cpl[:])
            consts = cpool.tile([P, TILES + 3], dt, tag="consts")
            nc.gpsimd.dma_start(consts[:], d_consts[:])
            eta_in = cpool.tile([P, 3 * TILES], dt, tag="eta_in")
            nc.gpsimd.dma_start(eta_in[:], d_eta0[:])

            toc16 = consts[:, 0:TILES]
            sA_ap = consts[:, TILES:TILES + 1]
            sK_ap = consts[:, TILES + 1:TILES + 2]
            sT_ap = consts[:, TILES + 2:TILES + 3]
            cpKL = cpl[:, 0:TILES]
            cpT0L = cpl[:, TILES: 2 * TILES]

            # ---- initial eta state + derived tiles ----
            etaA = spool.tile([P, TILES], dt, tag="etaA")
            etaK = spool.tile([P, TILES], dt, tag="etaK")
            etaT0 = spool.tile([P, TILES], dt, tag="etaT0")
            nc.vector.tensor_copy(etaA[:], eta_in[:, 0:TILES])
            nc.vector.tensor_copy(etaK[:], eta_in[:, TILES: 2 * TILES])
            nc.vector.tensor_copy(etaT0[:], eta_in[:, 2 * TILES: 3 * TILES])

            def make_derived(eA, eK, eT):
                # kn[:, 2t] = (k*t0)_t, kn[:, 2t+1] = (-k)_t  (bf16), then
                # transpose so tile t's arg-matmul rhs is knT[2t:2t+2, :]
                kn = spool.tile([P, 2 * TILES], bf, tag="kn")
                nc.vector.tensor_tensor(kn[:, 0:2 * TILES:2], eK[:], eT[:],
                                        Alu.mult)
                nc.vector.tensor_scalar_mul(kn[:, 1:2 * TILES:2], eK[:], -1.0)
                knt_ps = ps_k.tile([2 * TILES, P], bf, tag="kntp")
                nc.tensor.transpose(knt_ps[:], kn[:], ident[:])
                knT = spool.tile([2 * TILES, P], bf, tag="knT")
                nc.scalar.copy(knT[:], knt_ps[:])
                a2c = spool.tile([P, TILES], dt, tag="a2c")
                nc.vector.tensor_tensor(a2c[:], eA[:], toc16, Alu.mult)
                return knT, a2c

            knT, a2c = make_derived(etaA, etaK, etaT0)

            for it in range(MAX_ITER):
                accGA = mpool.tile([P, TILES], dt, tag="accGA")
                accU = mpool.tile([P, TILES], dt, tag="accU")
                accV = mpool.tile([P, TILES], dt, tag="accV")

                for t in range(TILES):
                    # argT[v,p] = kt0_p - k_p*tsh_v via rank-2 matmul:
                    # lhsT = [ones; tsh] chunk, rhs = knT[2t:2t+2, :]
                    argp = ps_t.tile([P, TOS], dt, tag="argp")
                    for c in range(4):
                        blk = 4 * t + c
                        nc.tensor.matmul(
                            argp[:, c * P:(c + 1) * P],
                            argw[:, blk * P:(blk + 1) * P],
                            knT[:],
                            start=True, stop=True,
                        )
                    # s1T = sigmoid(argT)  (PSUM -> SBUF, bf16 out)
                    s1T = wpool.tile([P, TOS], bf, tag="s1T")
                    nc.scalar.activation(s1T[:], argp[:], Act.Sigmoid)
                    # sdT = s1T*(1-s1T)
                    sdT = wpool.tile([P, TOS], bf, tag="sdT")
                    sdacc = wpool.tile([P, 1], dt, tag="sdacc")
                    nc.vector.affine_mul_reduce(
                        sdT[:], sdacc[:], s1T[:], s1T[:], -1.0, 1.0,
                    )

                    # q = M2 @ s1 -> [128p, 64]; qd|qdv = (M2|M2V) @ sd -> [128p, 128]
                    qq = ps_q.tile([P, 3 * T], dt, tag="qq")
                    for c in range(4):
                        nc.tensor.matmul(
                            qq[:, 0:T], s1T[:, c * P:(c + 1) * P],
                            m2t[:, c * T:(c + 1) * T],
                            start=(c == 0), stop=(c == 3),
                        )
                    for c in range(4):
                        nc.tensor.matmul(
                            qq[:, T: 3 * T], sdT[:, c * P:(c + 1) * P],
                            muv[:, c * 2 * T:(c + 1) * 2 * T],
                            start=(c == 0), stop=(c == 3),
                        )
                    # single PSUM->SBUF copy (bf16) for all of q|qd|qdv
                    qqs = wpool.tile([P, 3 * T], bf, tag="qqs")
                    nc.scalar.copy(qqs[:], qq[:])
                    q_ap = qqs[:, 0:T]
                    qd_ap = qqs[:, T: 2 * T]
                    qdv_ap = qqs[:, 2 * T: 3 * T]

                    # r2 = (2A/C_dc)*q - (2/C_dc)*ctc_dc
                    r2 = wpool.tile([P, T], dt, tag="r2")
                    nc.vector.affine_then_add(
                        r2[:], q_ap, nctc[:, t * T:(t + 1) * T],
                        a2c[:, t:t + 1], 0.0,
                    )
                    # dots: accGA[:,t] = sum r2*q ; accU ; accV   (seed 0;
                    # the cpa prior-term is added during the combine phase)
                    dsc = wpool.tile([P, 3 * T], dt, tag="dsc")
                    nc.vector.affine_mul_reduce(
                        dsc[:, 0:T], accGA[:, t:t + 1], q_ap, r2[:], 1.0, 0.0)
                    nc.vector.affine_mul_reduce(
                        dsc[:, T: 2 * T], accU[:, t:t + 1], qd_ap, r2[:], 1.0, 0.0)
                    nc.vector.affine_mul_reduce(
                        dsc[:, 2 * T: 3 * T], accV[:, t:t + 1], qdv_ap, r2[:], 1.0, 0.0)

                # ---- combine: eta <- eta - LR*grad (FD=16 ops) ----
                mA = mpool.tile([P, TILES], dt, tag="mA")
                nc.vector.tensor_scalar(mA[:], etaA[:], 0.0, -2.0 * LR,
                                        Alu.min, Alu.mult)
                ga2 = mpool.tile([P, TILES], dt, tag="ga2")
                nc.vector.tensor_tensor(ga2[:], accGA[:], cpa[:], Alu.add)
                tA = mpool.tile([P, TILES], dt, tag="tA")
                nc.vector.affine_then_add(tA[:], ga2[:], mA[:], -LR, 0.0)
                etaA2 = spool.tile([P, TILES], dt, tag="etaA")
                nc.vector.affine_then_add(etaA2[:], etaA[:], tA[:], sA_ap, 0.0)

                p1 = mpool.tile([P, TILES], dt, tag="p1")
                nc.vector.tensor_tensor(p1[:], etaA[:], accU[:], Alu.mult)
                p2 = mpool.tile([P, TILES], dt, tag="p2")
                nc.vector.tensor_tensor(p2[:], etaA[:], accV[:], Alu.mult)
                wk = mpool.tile([P, TILES], dt, tag="wk")
                nc.vector.tensor_tensor(wk[:], etaT0[:], p1[:], Alu.mult)
                zk = mpool.tile([P, TILES], dt, tag="zk")
                nc.vector.tensor_tensor(zk[:], wk[:], p2[:], Alu.subtract)
                mK = mpool.tile([P, TILES], dt, tag="mK")
                nc.vector.tensor_scalar(mK[:], etaK[:], 0.0, -2.0 * LR,
                                        Alu.min, Alu.mult)
                tK = mpool.tile([P, TILES], dt, tag="tK")
                nc.vector.affine_then_add(tK[:], zk[:], mK[:], -LR, 0.0)
                tK2 = mpool.tile([P, TILES], dt, tag="tK2")
                nc.vector.tensor_tensor(tK2[:], tK[:], cpKL, Alu.add)
                etaK2 = spool.tile([P, TILES], dt, tag="etaK")
                nc.vector.affine_then_add(etaK2[:], etaK[:], tK2[:], sK_ap, 0.0)

                w2 = mpool.tile([P, TILES], dt, tag="w2")
                nc.vector.tensor_tensor(w2[:], p1[:], etaK[:], Alu.mult)
                mT = mpool.tile([P, TILES], dt, tag="mT")
                nc.vector.tensor_scalar(mT[:], etaT0[:], 0.0, -2.0 * LR,
                                        Alu.min, Alu.mult)
                tT = mpool.tile([P, TILES], dt, tag="tT")
                nc.vector.affine_then_add(tT[:], w2[:], mT[:], -LR, 0.0)
                tT2 = mpool.tile([P, TILES], dt, tag="tT2")
                nc.vector.tensor_tensor(tT2[:], tT[:], cpT0L, Alu.add)
                etaT02 = spool.tile([P, TILES], dt, tag="etaT0")
                nc.vector.affine_then_add(etaT02[:], etaT0[:], tT2[:], sT_ap, 0.0)

                etaA, etaK, etaT0 = etaA2, etaK2, etaT02
                if it < MAX_ITER - 1:
                    knT, a2c = make_derived(etaA, etaK, etaT0)

            nc.gpsimd.dma_start(d_out[:, 0:TILES], etaA[:])
            nc.gpsimd.dma_start(d_out[:, TILES: 2 * TILES], etaK[:])
            nc.gpsimd.dma_start(d_out[:, 2 * TILES: 3 * TILES], etaT0[:])

    nc.finalize()
    _NC_CACHE["nc"] = nc
    return nc


# ---------------------------------------------------------------------------
# public entry point
# ---------------------------------------------------------------------------

def _make_in_maps(ctc, aif, time, eta_nn, lambda_reg):
    f32 = np.float32
    M2, M2V, tsh, ctc_dc, C_dc, creg = _preprocess(ctc, aif, time, eta_nn, lambda_reg)

    toc = 2.0 / C_dc
    sA, sK, sT0 = (1.0 - LR * creg).astype(np.float64)

    import ml_dtypes
    bf16 = ml_dtypes.bfloat16
    # argw[r, 128*(4t+c)+vv] = 1 if r==2t else tsh[128c+vv] if r==2t+1 else 0
    argw = np.zeros((2 * TILES, 4 * TILES * P), bf16)
    tshf = tsh.astype(np.float32)
    for t_ in range(TILES):
        for c_ in range(4):
            blk = 4 * t_ + c_
            argw[2 * t_, blk * P:(blk + 1) * P] = 1.0
            argw[2 * t_ + 1, blk * P:(blk + 1) * P] = tshf[c_ * P:(c_ + 1) * P]
    ident = np.eye(P, dtype=bf16)
    # m2t[vv, 64c+j] = M2[j, 128c+vv];  muv[vv, 128c+j'] = (M2|M2V)[j', 128c+vv]
    m2t = np.zeros((P, 4 * T), bf16)
    muv = np.zeros((P, 8 * T), bf16)
    for c in range(4):
        blk = M2[:, c * P:(c + 1) * P]       # [64,128]
        blkv = M2V[:, c * P:(c + 1) * P]
        m2t[:, c * T:(c + 1) * T] = blk.T
        muv[:, c * 2 * T: c * 2 * T + T] = blk.T
        muv[:, c * 2 * T + T: (c + 1) * 2 * T] = blkv.T

    consts = np.zeros((P, TILES + 3), f32)
    consts[:, 0:TILES] = toc
    consts[:, TILES] = sA
    consts[:, TILES + 1] = sK
    consts[:, TILES + 2] = sT0

    in_maps = []
    for m in range(N_CORES):
        rows = slice(m * ROWS_PER_CORE, (m + 1) * ROWS_PER_CORE)
        # ctc_dc[h, w, j]: tile t = local row, partition p = w
        cd = ctc_dc[rows]                     # [16, 128, 64]
        negctc2 = np.ascontiguousarray(
            (-toc * cd).transpose(1, 0, 2).reshape(P, TILES * T)).astype(f32)
        pr = eta_nn[0, :, rows, :].astype(np.float64)   # [3, 16, 128] (c, t, p)
        eta0 = np.ascontiguousarray(
            pr.transpose(2, 0, 1).reshape(P, 3 * TILES)).astype(f32)
        cpa = np.ascontiguousarray((-creg[0] * pr[0]).T).astype(f32)  # [128,16]
        cpl = np.zeros((P, 2 * TILES), f32)
        cpl[:, 0:TILES] = (LR * creg[1] * pr[1]).T
        cpl[:, TILES: 2 * TILES] = (LR * creg[2] * pr[2]).T
        in_maps.append({
            "argw": argw, "ident": ident, "m2t": m2t, "muv": muv,
            "negctc2": negctc2, "eta0": eta0, "cpa": cpa, "cpl": cpl,
            "consts": consts,
        })
    return in_maps


def kernel(ctc, aif, time, seg, eta_nn, lambda_reg):
    from concourse.bass_utils import run_bass_kernel_spmd

    ctc = np.asarray(ctc)
    aif = np.asarray(aif)
    time = np.asarray(time)
    eta_nn = np.asarray(eta_nn)
    lambda_reg = np.asarray(lambda_reg)

    in_maps = _make_in_maps(ctc, aif, time, eta_nn, lambda_reg)
    nc = _build_nc()
    res = run_bass_kernel_spmd(nc, in_maps, list(range(N_CORES)))

    out = np.zeros((1, 3, H, W), np.float32)
    for m in range(N_CORES):
        rows = slice(m * ROWS_PER_CORE, (m + 1) * ROWS_PER_CORE)
        arr = res.results[m]["out"]                  # [128, 48]
        out[0, :, rows, :] = arr.reshape(P, 3, TILES).transpose(1, 2, 0)
    return out
